# revision 1
# baseline (speedup 1.0000x reference)
import numpy as np
import jax
import jax.numpy as jnp
from functools import partial

# nn_LGGNet: B=64, N=62, D=4, T=512. Shard T across 8 cores (BN stats are
# per-timestep over (batch, feature), so T-sharding needs no cross-core comms).
B, N, D, T = 64, 62, 4, 512
NCORES = 8
EPS = 1e-5


def _bn(h, gamma, beta):
    mean = h.mean(axis=(1, 3), keepdims=True)
    var = h.var(axis=(1, 3), keepdims=True)
    return (h - mean) * jax.lax.rsqrt(var + EPS) * gamma[None, None, :, None] \
        + beta[None, None, :, None]


def _shard_fn(xt, local_w, local_b, global_adj, gcn_w, gcn_b,
              bn1_gamma, bn1_beta, bn2_gamma, bn2_beta):
    # xt: (T_loc, B, N, D)
    out = jax.nn.relu(xt * local_w[None, None] - local_b[None])
    s = jnp.einsum('tbnd,tbmd->tbnm', out, out)
    g = global_adj + global_adj.T
    adj = jax.nn.relu(s * g) + jnp.eye(N, dtype=xt.dtype)
    rowsum = adj.sum(-1)
    rowsum = jnp.where(rowsum == 0, 1.0, rowsum)
    d = rowsum ** -0.5
    adj = adj * d[..., :, None] * d[..., None, :]
    h = _bn(out, bn1_gamma, bn1_beta)
    h = h @ gcn_w - gcn_b[None]
    h = jax.nn.relu(jnp.einsum('tbnm,tbmd->tbnd', adj, h))
    h = _bn(h, bn2_gamma, bn2_beta)
    return h  # (T_loc, B, N, D)


_compiled = None


def _get_compiled():
    global _compiled
    if _compiled is None:
        devs = jax.devices()[:NCORES]
        fn = jax.pmap(_shard_fn, axis_name='i', devices=devs,
                      in_axes=(0, None, None, None, None, None,
                               None, None, None, None))
        _compiled = fn
    return _compiled


def kernel(x, local_w, local_b, global_adj, gcn_w, gcn_b,
           bn1_gamma, bn1_beta, bn2_gamma, bn2_beta):
    x = np.asarray(x, dtype=np.float32)
    # (B,N,D,T) -> (T,B,N,D) -> (8, T/8, B, N, D)
    xt = np.moveaxis(x, -1, 0)
    xt_sh = xt.reshape(NCORES, T // NCORES, B, N, D)
    fn = _get_compiled()
    h = fn(xt_sh, jnp.asarray(local_w), jnp.asarray(local_b),
           jnp.asarray(global_adj), jnp.asarray(gcn_w), jnp.asarray(gcn_b),
           jnp.asarray(bn1_gamma), jnp.asarray(bn1_beta),
           jnp.asarray(bn2_gamma), jnp.asarray(bn2_beta))
    h = np.asarray(h)                      # (8, T/8, B, N, D)
    h = h.reshape(T, B, N, D)
    return np.moveaxis(h, 0, -1).astype(np.float32)   # (B,N,D,T)



# revision 3
# speedup vs baseline: 1.3251x; 1.3251x over previous
import hashlib
import numpy as np
import jax
import jax.numpy as jnp
import ml_dtypes

# nn_LGGNet: B=64, N=62, D=4, T=512.
# Shard B across 8 cores (zero-copy host reshape); BN stats over (batch,
# feature) per timestep are computed with a psum over cores. Data crosses
# the (slow) axon tunnel as bf16, T-chunked so uploads, compute, and
# downloads overlap (the tunnel is full-duplex).
B, N, D, T = 64, 62, 4, 512
NCORES = 8
B_LOC = B // NCORES
EPS = 1e-5
NCHUNKS = 4
TC = T // NCHUNKS
BF16 = ml_dtypes.bfloat16


def _bn_psum(h, gamma, beta):
    # h: (Tc, B_loc, N, D); stats over global batch (psum) and feature dims
    s1 = h.sum(axis=(1, 3), keepdims=True)
    s2 = (h * h).sum(axis=(1, 3), keepdims=True)
    s1 = jax.lax.psum(s1, 'i')
    s2 = jax.lax.psum(s2, 'i')
    cnt = B * D
    mean = s1 / cnt
    var = s2 / cnt - mean * mean
    return (h - mean) * jax.lax.rsqrt(var + EPS) * gamma[None, None, :, None] \
        + beta[None, None, :, None]


def _shard_fn(xb, local_w, local_b, global_adj, gcn_w, gcn_b,
              bn1_gamma, bn1_beta, bn2_gamma, bn2_beta):
    # xb: (B_loc, N, D, Tc) bf16
    x = xb.astype(jnp.float32)
    xt = jnp.moveaxis(x, -1, 0)                      # (Tc, B_loc, N, D)
    out = jax.nn.relu(xt * local_w[None, None] - local_b[None])
    s = jnp.einsum('tbnd,tbmd->tbnm', out, out)
    g = global_adj + global_adj.T
    adj = jax.nn.relu(s * g) + jnp.eye(N, dtype=x.dtype)
    rowsum = adj.sum(-1)
    rowsum = jnp.where(rowsum == 0, 1.0, rowsum)
    d = rowsum ** -0.5
    adj = adj * d[..., :, None] * d[..., None, :]
    h = _bn_psum(out, bn1_gamma, bn1_beta)
    h = h @ gcn_w - gcn_b[None]
    h = jax.nn.relu(jnp.einsum('tbnm,tbmd->tbnd', adj, h))
    h = _bn_psum(h, bn2_gamma, bn2_beta)
    h = jnp.moveaxis(h, 0, -1)                       # (B_loc, N, D, Tc)
    return h.astype(jnp.bfloat16)


_compiled = None
_param_cache = {}


def _get_compiled():
    global _compiled
    if _compiled is None:
        devs = jax.devices()[:NCORES]
        _compiled = jax.pmap(_shard_fn, axis_name='i', devices=devs, in_axes=0)
    return _compiled


def _cached_params(params):
    key = hashlib.sha256(b"".join(np.ascontiguousarray(p).tobytes()
                                  for p in params)).hexdigest()
    hit = _param_cache.get(key)
    if hit is not None:
        return hit
    devs = jax.devices()[:NCORES]
    dev_params = [jax.device_put_replicated(jnp.asarray(p), devs)
                  for p in params]
    _param_cache.clear()
    _param_cache[key] = dev_params
    return dev_params


def kernel(x, local_w, local_b, global_adj, gcn_w, gcn_b,
           bn1_gamma, bn1_beta, bn2_gamma, bn2_beta):
    fn = _get_compiled()
    params = _cached_params([local_w, local_b, global_adj, gcn_w, gcn_b,
                             bn1_gamma, bn1_beta, bn2_gamma, bn2_beta])

    x = np.asarray(x, dtype=np.float32)
    xb = x.astype(BF16)                              # (B, N, D, T)
    xsh = xb.reshape(NCORES, B_LOC, N, D, T)         # zero-copy view

    results = []
    for k in range(NCHUNKS):
        chunk = np.ascontiguousarray(xsh[..., k * TC:(k + 1) * TC])
        results.append(fn(chunk, *params))           # async dispatch

    out = np.empty((B, N, D, T), dtype=np.float32)
    for k, r in enumerate(results):
        rk = np.asarray(r)                           # (8, B_loc, N, D, Tc) bf16
        out[..., k * TC:(k + 1) * TC] = rk.reshape(B, N, D, TC)
    return out


# revision 5
# speedup vs baseline: 2.2675x; 1.7112x over previous
import hashlib
import concurrent.futures as _cf
import numpy as np
import jax
import jax.numpy as jnp
import ml_dtypes
from jax.sharding import Mesh, NamedSharding, PartitionSpec as P
from jax.experimental.shard_map import shard_map

# nn_LGGNet: B=64, N=62, D=4, T=512.
# The 8 NeuronCores sit behind a slow (~70MB/s per direction, full-duplex)
# tunnel, so wall time is transfer-dominated. Strategy:
#   - bf16 on the wire both ways (tolerance 2e-2 >> bf16 error)
#   - shard B across cores (zero-copy host reshape); BN stats use psum
#   - thread-parallel device_put/np.asarray (single-thread dispatch
#     serializes ~90ms/op of fixed cost; threads hide it)
#   - T-chunked pipeline so uploads, compute, and downloads overlap
B, N, D, T = 64, 62, 4, 512
NCORES = 8
B_LOC = B // NCORES
EPS = 1e-5
NCHUNKS = 2
TC = T // NCHUNKS
BF16 = ml_dtypes.bfloat16


def _bn_psum(h, gamma, beta):
    # h: (Tc, B_loc, N, D); stats over global batch (psum) and feature dims
    s1 = h.sum(axis=(1, 3), keepdims=True)
    s2 = (h * h).sum(axis=(1, 3), keepdims=True)
    s1 = jax.lax.psum(s1, 'i')
    s2 = jax.lax.psum(s2, 'i')
    cnt = B * D
    mean = s1 / cnt
    var = s2 / cnt - mean * mean
    return (h - mean) * jax.lax.rsqrt(var + EPS) * gamma[None, None, :, None] \
        + beta[None, None, :, None]


def _shard_fn(xb, local_w, local_b, global_adj, gcn_w, gcn_b,
              bn1_gamma, bn1_beta, bn2_gamma, bn2_beta):
    # xb: (B_loc, N, D, Tc) bf16
    x = xb.astype(jnp.float32)
    xt = jnp.moveaxis(x, -1, 0)                      # (Tc, B_loc, N, D)
    out = jax.nn.relu(xt * local_w[None, None] - local_b[None])
    s = jnp.einsum('tbnd,tbmd->tbnm', out, out)
    g = global_adj + global_adj.T
    adj = jax.nn.relu(s * g) + jnp.eye(N, dtype=x.dtype)
    rowsum = adj.sum(-1)
    rowsum = jnp.where(rowsum == 0, 1.0, rowsum)
    d = rowsum ** -0.5
    adj = adj * d[..., :, None] * d[..., None, :]
    h = _bn_psum(out, bn1_gamma, bn1_beta)
    h = h @ gcn_w - gcn_b[None]
    h = jax.nn.relu(jnp.einsum('tbnm,tbmd->tbnd', adj, h))
    h = _bn_psum(h, bn2_gamma, bn2_beta)
    h = jnp.moveaxis(h, 0, -1)                       # (B_loc, N, D, Tc)
    return h.astype(jnp.bfloat16)


_state = {}


def _get_state():
    if not _state:
        devs = jax.devices()[:NCORES]
        mesh = Mesh(np.array(devs), ('i',))
        xspec = P('i')
        pspec = P()
        fn = jax.jit(shard_map(
            _shard_fn, mesh=mesh,
            in_specs=(xspec,) + (pspec,) * 9,
            out_specs=xspec, check_rep=False))
        _state['devs'] = devs
        _state['mesh'] = mesh
        _state['fn'] = fn
        _state['xsharding'] = NamedSharding(mesh, xspec)
        _state['psharding'] = NamedSharding(mesh, pspec)
        _state['up_pool'] = _cf.ThreadPoolExecutor(NCORES)
        _state['down_pool'] = _cf.ThreadPoolExecutor(NCORES)
        _state['param_cache'] = {}
    return _state


def _cached_params(st, params):
    key = hashlib.sha256(b"".join(np.ascontiguousarray(p).tobytes()
                                  for p in params)).hexdigest()
    cache = st['param_cache']
    hit = cache.get(key)
    if hit is not None:
        return hit
    dev_params = [jax.device_put(jnp.asarray(p), st['psharding'])
                  for p in params]
    cache.clear()
    cache[key] = dev_params
    return dev_params


def kernel(x, local_w, local_b, global_adj, gcn_w, gcn_b,
           bn1_gamma, bn1_beta, bn2_gamma, bn2_beta):
    st = _get_state()
    devs, fn = st['devs'], st['fn']
    params = _cached_params(st, [local_w, local_b, global_adj, gcn_w, gcn_b,
                                 bn1_gamma, bn1_beta, bn2_gamma, bn2_beta])

    x = np.asarray(x, dtype=np.float32)
    xb = x.astype(BF16)                              # (B, N, D, T) bf16
    xsh = xb.reshape(NCORES, B_LOC, N, D, T)         # zero-copy view

    def _up(args):
        c, k = args
        shard = np.ascontiguousarray(xsh[c, ..., k * TC:(k + 1) * TC])
        return jax.device_put(shard, devs[c])

    gshape = (B, N, D, TC)

    def _down(args):
        k, shard = args
        c = shard.index[0].start // B_LOC
        return k, c, np.asarray(shard.data)

    down_futs = []
    for k in range(NCHUNKS):
        puts = list(st['up_pool'].map(_up, [(c, k) for c in range(NCORES)]))
        garr = jax.make_array_from_single_device_arrays(
            gshape, st['xsharding'], puts)
        res = fn(garr, *params)                      # async dispatch
        for sh in res.addressable_shards:
            down_futs.append(st['down_pool'].submit(_down, (k, sh)))

    out = np.empty((B, N, D, T), dtype=np.float32)
    osh = out.reshape(NCORES, B_LOC, N, D, T)
    for f in down_futs:
        k, c, arr = f.result()
        osh[c, ..., k * TC:(k + 1) * TC] = arr
    return out
